# revision 2
# baseline (speedup 1.0000x reference)
"""Trainium2 Bass kernel for a ViT-style attention block + classifier head.

Reference computation (per batch b of 4, N=2048 tokens, C=768, 12 heads x 64):
    qkv  = x @ w_qkv                         [B,N,3C]
    attn = softmax(q k^T / 8)                per head
    out  = (attn @ v) reassembled            [B,N,C]
    out  = out @ w_proj + b_proj
    out  = out @ w_head + b_head             [B,N,1000]
    return max over N                        [B,1000]

Sharding: 8 cores = 4 batches x 2 query-halves (1024 queries each).
Each core computes K/V for its full batch (duplicated within the pair),
attention for its query half, then a fused (w_proj @ w_head) classifier
matmul and a local max over its 1024 queries -> [1000] per core.
Host combines with np.maximum and adds the fused bias afterwards
(max is invariant to adding a per-row constant).

Key performance structure (cost-model driven):
  * Scores (q k^T) run as fp8e4 DoubleRow matmuls (0.5 cycles/row): q,k are
    scaled x8 on the host (folded out of the softmax via the exp activation
    scale), evacuated from PSUM as fp8, and the DoubleRow second contraction
    lane points at a zeroed column block so no partition repacking is needed.
  * attn@v is computed transposed: out[q, d] accumulates over key chunks in
    PSUM with only 65 streamed columns per matmul (64 v-dims + a ones column
    that yields the softmax denominator), queries on the partition axis.
    16 accumulators (2 heads x 8 query chunks) pack into 3 PSUM banks as
    130-column slots; the first matmul in each bank carries start=True
    (PSUM zero regions are bank-granular), the rest rely on the lazy
    bank-wide zero.
  * Softmax normalize is a per-partition reciprocal + tensor_scalar multiply
    on DVE writing [q, chan]-layout tiles; a cheap PE transpose (identity
    permutation matmul) restores [chan, q] for the classifier.
  * Exps run 1024-wide on ScalarE from 2-bank PSUM score tiles to amortize
    the per-instruction access latency; ScalarE is the bottleneck engine.

All matmuls accumulate in fp32 PSUM; qkv/attn@v/classifier operands are
bf16, scores operands fp8e4 (quantization feeds only the exp argument,
damped by the 1/8 softmax scale).
"""

import sys

for _p in ("/opt/trn_rl_repo", "/root/.axon_site/_ro/trn_rl_repo"):
    if _p not in sys.path:
        sys.path.append(_p)

import numpy as np
import ml_dtypes

import concourse.bacc as bacc
import concourse.mybir as mybir
from concourse.tile import TileContext
from concourse.bass_utils import run_bass_kernel_spmd

BF16 = mybir.dt.bfloat16
FP8 = mybir.dt.float8e4
F32 = mybir.dt.float32
DR = mybir.MatmulPerfMode.DoubleRow

B, N, C = 4, 2048, 768
HEADS, HD = 12, 64
NUM_CLASSES = 1000
SCALE = HD ** (-0.5)
QK_PRESCALE = 8.0          # host-folded into w_qkv's q/k columns

NQ = 1024          # queries per core
KC = N // 128      # 16 key chunks
CC = C // 128      # 6 contraction chunks
PAIRS = HEADS // 2  # head-pair tiles (2 x 64 partitions)
NCLS = NUM_CLASSES

_CACHE = {}


def _build():
    nc = bacc.Bacc("TRN2", target_bir_lowering=False)

    # xT arrives key-rotated per core so that columns 0:NQ are always this
    # core's query rows (attention is invariant to key order; the final max
    # is invariant to query order).
    xT_d = nc.dram_tensor("xT", [C, N], BF16, kind="ExternalInput")
    wqkv_d = nc.dram_tensor("wqkv", [C, 3 * C], BF16, kind="ExternalInput")
    wf_d = nc.dram_tensor("wf", [C, NCLS], BF16, kind="ExternalInput")
    ident_d = nc.dram_tensor("ident", [128, 128], BF16, kind="ExternalInput")
    out_d = nc.dram_tensor("out", [128, NCLS], F32, kind="ExternalOutput")

    EXP = mybir.ActivationFunctionType.Exp
    ESCALE = SCALE / (QK_PRESCALE * QK_PRESCALE)

    with TileContext(nc) as tc:
        with (
            tc.tile_pool(name="wpool", bufs=1) as wpool,
            tc.tile_pool(name="xpool", bufs=1) as xpool,
            tc.tile_pool(name="qk8p", bufs=1) as qk8p,
            tc.tile_pool(name="vp", bufs=1) as vp,
            tc.tile_pool(name="ep", bufs=6) as ep,
            tc.tile_pool(name="outp", bufs=1) as outp,
            tc.tile_pool(name="small", bufs=4) as smallp,
            tc.tile_pool(name="lg", bufs=1) as lgp,
            # PSUM: 2 x [128,1024] rotating score/projection tiles (4 banks),
            # 3 x [128,512] attn@v accumulator banks, 1 x transpose bank.
            tc.tile_pool(name="ps", bufs=2, space="PSUM") as psp,
            tc.tile_pool(name="av", bufs=3, space="PSUM") as avp,
            tc.tile_pool(name="tp", bufs=1, space="PSUM") as tpp,
        ):
            # ---- static SBUF tiles ----
            wqkv = [wpool.tile([128, 3 * C], BF16, tag="wqkv", name="wqkv_sb", bufs=CC) for _ in range(CC)]
            wf = [wpool.tile([128, NCLS], BF16, tag="wf", name="wf_sb", bufs=CC) for _ in range(CC)]
            xT = [xpool.tile([128, N], BF16, tag="xT", name="xT_sb", bufs=CC) for _ in range(CC)]
            ident = wpool.tile([128, 128], BF16, tag="ident", name="ident_sb", bufs=1)
            # Per pair: fp8 q/k with a zeroed DoubleRow companion lane.
            # cols 0:NQ q values, NQ:2NQ q zero lane, 2NQ:2NQ+N k values,
            # 2NQ+N: k zero lane.  (2*NQ + 2*N = 6144 cols)
            qk8 = [qk8p.tile([128, 2 * NQ + 2 * N], FP8, tag="qk8", name="qk8_sb", bufs=PAIRS)
                   for _ in range(PAIRS)]
            # v with a ones column appended per head: [128, 12*65]
            v65 = [vp.tile([128, HEADS * (HD + 1)], BF16, tag="v65", name="v65_sb", bufs=KC)
                   for _ in range(KC)]
            # normalized attention output, queries on partitions: col h*64+d
            out_qc = [outp.tile([128, C], BF16, tag="oqc", name="oqc_sb", bufs=NQ // 128)
                      for _ in range(NQ // 128)]
            # transposed back: channels on partitions
            outT = [outp.tile([128, NQ], BF16, tag="outT", name="outT_sb", bufs=CC) for _ in range(CC)]
            lgmax = lgp.tile([128, NCLS], F32, tag="lgmax")

            # ---- input DMA (ordered for the pipeline lead-in) ----
            for c in range(CC):
                sl = slice(c * 128, (c + 1) * 128)
                nc.sync.dma_start(out=xT[c][:, 0:NQ], in_=xT_d[sl, 0:NQ])
                nc.sync.dma_start(out=wqkv[c][:, 0:C], in_=wqkv_d[sl, 0:C])
            nc.sync.dma_start(out=ident[:], in_=ident_d[:, :])
            for c in range(CC):
                sl = slice(c * 128, (c + 1) * 128)
                nc.sync.dma_start(out=xT[c][:, NQ:N], in_=xT_d[sl, NQ:N])
                nc.sync.dma_start(out=wqkv[c][:, C:2 * C], in_=wqkv_d[sl, C:2 * C])
            for c in range(CC):
                sl = slice(c * 128, (c + 1) * 128)
                nc.sync.dma_start(out=wqkv[c][:, 2 * C:3 * C],
                                  in_=wqkv_d[sl, 2 * C:3 * C])
            for c in range(CC):
                sl = slice(c * 128, (c + 1) * 128)
                nc.sync.dma_start(out=wf[c][:], in_=wf_d[sl, :])

            # DoubleRow zero lanes (GPSIMD is otherwise idle here)
            for p in range(PAIRS):
                nc.gpsimd.memset(qk8[p][:, NQ:2 * NQ], 0.0)
                nc.gpsimd.memset(qk8[p][:, 2 * NQ + N:], 0.0)

            # ---- per-pair q/k projection -> fp8 (with zero DR lane) ----
            def q_unit(p):
                ps = psp.tile([128, 1024], F32, tag="ps", name="ps")
                for c in range(CC):
                    for s0 in (0, 512):
                        nc.tensor.matmul(
                            ps[:, s0:s0 + 512],
                            lhsT=wqkv[c][:, p * 128:(p + 1) * 128],
                            rhs=xT[c][:, s0:s0 + 512],
                            start=(c == 0), stop=(c == CC - 1))
                nc.vector.tensor_copy(out=qk8[p][:, 0:NQ], in_=ps[:])

            def k_unit(p, u):
                # keys u*1024 .. (u+1)*1024
                ps = psp.tile([128, 1024], F32, tag="ps", name="ps")
                for c in range(CC):
                    for s0 in (0, 512):
                        nc.tensor.matmul(
                            ps[:, s0:s0 + 512],
                            lhsT=wqkv[c][:, C + p * 128:C + (p + 1) * 128],
                            rhs=xT[c][:, u * 1024 + s0:u * 1024 + s0 + 512],
                            start=(c == 0), stop=(c == CC - 1))
                nc.vector.tensor_copy(
                    out=qk8[p][:, 2 * NQ + u * 1024:2 * NQ + (u + 1) * 1024], in_=ps[:])

            def qk_units(p):
                return [lambda p=p: q_unit(p),
                        lambda p=p: k_unit(p, 0),
                        lambda p=p: k_unit(p, 1)]

            def v_unit(kc, p):
                """v65[kc] for pair p's two heads (+ their ones columns)."""
                ps = psp.tile([128, 1024], F32, tag="ps", name="ps")
                for c in range(CC):
                    nc.tensor.matmul(
                        ps[:, 0:2 * HD], lhsT=xT[c][:, kc * 128:(kc + 1) * 128],
                        rhs=wqkv[c][:, 2 * C + 2 * p * HD:2 * C + (2 * p + 2) * HD],
                        start=(c == 0), stop=(c == CC - 1))
                vdst = v65[kc][:].rearrange("p (h d) -> p h d", d=HD + 1)
                nc.vector.memset(vdst[:, 2 * p:2 * p + 2, HD:HD + 1], 1.0)
                nc.vector.tensor_copy(
                    out=vdst[:, 2 * p:2 * p + 2, 0:HD],
                    in_=ps[:, 0:2 * HD].rearrange("p (h d) -> p h d", d=HD))

            # av accumulator geometry: 8 slots of 130 cols over 3 banks
            # bank 0: qc 0..2, bank 1: qc 3..5, bank 2: qc 6..7
            def av_slot(qc):
                return qc // 3, (qc % 3) * 130

            def attention_pair(p, post_fillers):
                avb = [avp.tile([128, 512], F32, tag="av", name="av") for _ in range(3)]
                qv = qk8[p][:, 0:2 * NQ].rearrange("p (i n) -> p i n", i=2)
                kv = qk8[p][:, 2 * NQ:].rearrange("p (i n) -> p i n", i=2)
                for kc in range(KC):
                    v_unit(kc, p)
                    es = []
                    for h in (0, 1):
                        rows = slice(h * HD, (h + 1) * HD)
                        st = psp.tile([128, 1024], F32, tag="ps", name="ps")
                        for s0 in (0, 512):
                            nc.tensor.matmul(
                                st[:, s0:s0 + 512],
                                lhsT=kv[rows, :, kc * 128:(kc + 1) * 128],
                                rhs=qv[rows, :, s0:s0 + 512],
                                start=True, stop=True, perf_mode=DR)
                        e = ep.tile([128, 1024], BF16, tag="e", name="e")
                        nc.scalar.activation(out=e[:], in_=st[:], func=EXP,
                                             scale=ESCALE)
                        es.append(e)
                    for qc in range(NQ // 128):
                        b, col = av_slot(qc)
                        for h in (0, 1):
                            # first matmul of each bank at kc==0 zeroes the
                            # whole bank (PSUM zero regions are 2KB); the
                            # last one per bank at kc==15 closes the group.
                            first = kc == 0 and col == 0 and h == 0
                            last = (kc == KC - 1 and h == 1
                                    and (qc % 3 == 2 or qc == NQ // 128 - 1))
                            nc.tensor.matmul(
                                avb[b][:, col + h * 65:col + h * 65 + 65],
                                lhsT=es[h][:, qc * 128:(qc + 1) * 128],
                                rhs=v65[kc][:, (2 * p + h) * 65:(2 * p + h) * 65 + 65],
                                start=first, stop=last, skip_group_check=not (first or last))
                    for f in post_fillers.get(kc, ()):
                        f()
                # normalize: per (qc, h): out_qc[qc][:, (2p+h)*64 :+64]
                #   = av[:, slot+h*65 : +64] * (1 / av[:, slot+h*65+64])
                for qc in range(NQ // 128):
                    b, col = av_slot(qc)
                    r = smallp.tile([128, 2], F32, tag="rcp", name="rcp")
                    den = avb[b][:, col + 64:col + 130:65]
                    nc.vector.reciprocal_approx_fast(out=r[:], in_=den)
                    for h in (0, 1):
                        nc.vector.tensor_scalar_mul(
                            out=out_qc[qc][:, (2 * p + h) * HD:(2 * p + h + 1) * HD],
                            in0=avb[b][:, col + h * 65:col + h * 65 + 64],
                            scalar1=r[:, h:h + 1])
                # transpose this pair's 128-col block back to [chan, q]
                for qc in range(NQ // 128):
                    tp = tpp.tile([128, 128], BF16, tag="tp", name="tp")
                    nc.tensor.transpose(
                        tp[:], in_=out_qc[qc][:, p * 128:(p + 1) * 128],
                        identity=ident[:])
                    nc.vector.tensor_copy(
                        out=outT[p][:, qc * 128:(qc + 1) * 128], in_=tp[:])

            # ---- schedule ----
            for f in qk_units(0):
                f()
            for p in range(PAIRS):
                post = {}
                if p + 1 < PAIRS:
                    for i, f in enumerate(qk_units(p + 1)):
                        post.setdefault(3 + 4 * i, []).append(f)
                attention_pair(p, post)

            # ---- fused classifier head + max over queries ----
            for qc in range(NQ // 128):
                ps = psp.tile([128, 1024], F32, tag="ps", name="ps")
                for s0 in (0, 512):
                    sw = min(512, NCLS - s0)
                    for c in range(CC):
                        nc.tensor.matmul(
                            ps[:, s0:s0 + sw],
                            lhsT=outT[c][:, qc * 128:(qc + 1) * 128],
                            rhs=wf[c][:, s0:s0 + sw],
                            start=(c == 0), stop=(c == CC - 1))
                if qc == 0:
                    nc.vector.tensor_copy(out=lgmax[:], in_=ps[:, 0:NCLS])
                else:
                    nc.vector.tensor_max(out=lgmax[:], in0=ps[:, 0:NCLS],
                                         in1=lgmax[:])

            # final 128-way partition max happens on the host
            nc.sync.dma_start(out=out_d[:, :], in_=lgmax[:])

    nc.compile()
    return nc


def _prep_inputs(x, w_qkv, w_proj, b_proj, w_head, b_head):
    bf = ml_dtypes.bfloat16
    x = np.asarray(x, dtype=np.float32)
    w_qkv = np.asarray(w_qkv, np.float32).copy()
    # fold the fp8 prescale into the q/k weight columns
    w_qkv[:, 0:2 * C] *= QK_PRESCALE
    wf = (np.asarray(w_proj, np.float64) @ np.asarray(w_head, np.float64))
    wf = wf.astype(np.float32)
    b_const = (np.asarray(b_proj, np.float32) @ np.asarray(w_head, np.float32)
               + np.asarray(b_head, np.float32))

    wqkv_b = np.ascontiguousarray(w_qkv.astype(bf))
    wf_b = np.ascontiguousarray(wf.astype(bf))
    ident = np.eye(128, dtype=bf)
    in_maps = []
    for core in range(8):
        b, half = core // 2, core % 2
        xb = x[b] if half == 0 else np.concatenate(
            [x[b, NQ:], x[b, :NQ]], axis=0)   # rotate keys: own queries first
        xTb = np.ascontiguousarray(xb.T.astype(bf))                # [768, 2048]
        in_maps.append({"xT": xTb, "wqkv": wqkv_b, "wf": wf_b, "ident": ident})
    return in_maps, b_const


def kernel(x, w_qkv, w_proj, b_proj, w_head, b_head):
    if "nc" not in _CACHE:
        _CACHE["nc"] = _build()
    nc = _CACHE["nc"]

    in_maps, b_const = _prep_inputs(x, w_qkv, w_proj, b_proj, w_head, b_head)
    res = run_bass_kernel_spmd(nc, in_maps, core_ids=list(range(8)))

    out = np.empty((B, NUM_CLASSES), np.float32)
    for b in range(B):
        lo = res.results[2 * b]["out"].max(axis=0)
        hi = res.results[2 * b + 1]["out"].max(axis=0)
        out[b] = np.maximum(lo, hi)[:NUM_CLASSES] + b_const
    return out


if __name__ == "__main__":
    sys.path.insert(0, "/root/problem")
    import reference

    inputs = {k: np.asarray(v) for k, v in reference.setup_inputs().items()}
    expected = np.asarray(reference.reference(**inputs))
    actual = kernel(**inputs)
    num = np.linalg.norm(actual - expected)
    den = np.linalg.norm(expected)
    print("rel fro err:", num / den)


# revision 4
# speedup vs baseline: 1.1467x; 1.1467x over previous
"""Trainium2 Bass kernel for a ViT-style attention block + classifier head.

Reference computation (per batch b of 4, N=2048 tokens, C=768, 12 heads x 64):
    qkv  = x @ w_qkv                         [B,N,3C]
    attn = softmax(q k^T / 8)                per head
    out  = (attn @ v) reassembled            [B,N,C]
    out  = out @ w_proj + b_proj
    out  = out @ w_head + b_head             [B,N,1000]
    return max over N                        [B,1000]

Sharding: 8 cores = 4 batches x 2 query-halves (1024 queries each).
Each core computes K/V for its full batch (duplicated within the pair),
attention for its query half, then a fused (w_proj @ w_head) classifier
matmul and a local max over its 1024 queries -> [1000] per core.
Host combines with np.maximum and adds the fused bias afterwards
(max is invariant to adding a per-row constant).

Key performance structure (cost-model driven):
  * Scores (q k^T) run as fp8e4 DoubleRow matmuls (0.5 cycles/row): q,k are
    scaled x8 on the host (folded out of the softmax via the exp activation
    scale), evacuated from PSUM as fp8, and the DoubleRow second contraction
    lane points at a zeroed column block so no partition repacking is needed.
  * attn@v is computed transposed: out[q, d] accumulates over key chunks in
    PSUM with only 65 streamed columns per matmul (64 v-dims + a ones column
    that yields the softmax denominator), queries on the partition axis.
    16 accumulators (2 heads x 8 query chunks) pack into 3 PSUM banks as
    130-column slots; the first matmul in each bank carries start=True
    (PSUM zero regions are bank-granular), the rest rely on the lazy
    bank-wide zero.
  * The whole kernel is emitted as one software-pipelined stream: attn@v
    matmuls trail their scores/exp by LAG key-chunks so that by the time
    they reach the PE wait queue their exp dependency is already satisfied
    (the 4-deep wait queue otherwise blocks the PE sequencer), and
    normalize/transpose work for a pair is spread over the next pair's
    steps.
  * Softmax normalize is a per-partition reciprocal + tensor_scalar multiply
    on DVE writing [q, chan]-layout tiles; a cheap PE transpose (identity
    permutation matmul) restores [chan, q] for the classifier.
  * Exps run 1024-wide on ScalarE from 2-bank PSUM score tiles; ScalarE is
    the bottleneck engine, so everything else hides behind it.
"""

import sys

for _p in ("/opt/trn_rl_repo", "/root/.axon_site/_ro/trn_rl_repo"):
    if _p not in sys.path:
        sys.path.append(_p)

import numpy as np
import ml_dtypes

import concourse.bacc as bacc
import concourse.mybir as mybir
from concourse.tile import TileContext
from concourse.bass_utils import run_bass_kernel_spmd

BF16 = mybir.dt.bfloat16
FP8 = mybir.dt.float8e4
F32 = mybir.dt.float32
DR = mybir.MatmulPerfMode.DoubleRow

B, N, C = 4, 2048, 768
HEADS, HD = 12, 64
NUM_CLASSES = 1000
SCALE = HD ** (-0.5)
QK_PRESCALE = 8.0          # host-folded into w_qkv's q/k columns

NQ = 1024          # queries per core
KC = N // 128      # 16 key chunks
CC = C // 128      # 6 contraction chunks
PAIRS = HEADS // 2  # head-pair tiles (2 x 64 partitions)
NCLS = NUM_CLASSES
LAG = 3            # attn@v trails scores/exp by this many key chunks
VG = 3             # v-production group width in pairs

_CACHE = {}


def _build():
    nc = bacc.Bacc("TRN2", target_bir_lowering=False)

    # xT arrives key-rotated per core so that columns 0:NQ are always this
    # core's query rows (attention is invariant to key order; the final max
    # is invariant to query order).
    xT_d = nc.dram_tensor("xT", [C, N], BF16, kind="ExternalInput")
    wqkv_d = nc.dram_tensor("wqkv", [C, 3 * C], BF16, kind="ExternalInput")
    wf_d = nc.dram_tensor("wf", [C, NCLS], BF16, kind="ExternalInput")
    ident_d = nc.dram_tensor("ident", [128, 128], BF16, kind="ExternalInput")
    out_d = nc.dram_tensor("out", [128, NCLS], F32, kind="ExternalOutput")

    EXP = mybir.ActivationFunctionType.Exp
    ESCALE = SCALE / (QK_PRESCALE * QK_PRESCALE)

    with TileContext(nc) as tc:
        with (
            tc.tile_pool(name="wpool", bufs=1) as wpool,
            tc.tile_pool(name="xpool", bufs=1) as xpool,
            tc.tile_pool(name="qk8p", bufs=1) as qk8p,
            tc.tile_pool(name="vp", bufs=1) as vp,
            tc.tile_pool(name="ep", bufs=8) as ep,
            tc.tile_pool(name="outp", bufs=1) as outp,
            tc.tile_pool(name="small", bufs=4) as smallp,
            tc.tile_pool(name="lg", bufs=1) as lgp,
            # PSUM: 2 x [128,1024] rotating score/projection tiles (4 banks),
            # 3 x [128,512] attn@v accumulator banks, 1 bank shared by the
            # transpose staging tile and the v-production tile.
            tc.tile_pool(name="ps", bufs=2, space="PSUM") as psp,
            tc.tile_pool(name="av", bufs=3, space="PSUM") as avp,
            tc.tile_pool(name="tp", bufs=1, space="PSUM") as tpp,
        ):
            # ---- static SBUF tiles ----
            wqkv = [wpool.tile([128, 3 * C], BF16, tag="wqkv", name="wqkv_sb", bufs=CC) for _ in range(CC)]
            wf = [wpool.tile([128, NCLS], BF16, tag="wf", name="wf_sb", bufs=CC) for _ in range(CC)]
            xT = [xpool.tile([128, N], BF16, tag="xT", name="xT_sb", bufs=CC) for _ in range(CC)]
            ident = wpool.tile([128, 128], BF16, tag="ident", name="ident_sb", bufs=1)
            # Per pair: fp8 q/k with a zeroed DoubleRow companion lane.
            # cols 0:NQ q values, NQ:2NQ q zero lane, 2NQ:2NQ+N k values,
            # 2NQ+N: k zero lane.  (2*NQ + 2*N = 6144 cols)
            qk8 = [qk8p.tile([128, 2 * NQ + 2 * N], FP8, tag="qk8", name="qk8_sb", bufs=PAIRS)
                   for _ in range(PAIRS)]
            # v with a ones column appended per head: [128, 12*65]
            v65 = [vp.tile([128, HEADS * (HD + 1)], BF16, tag="v65", name="v65_sb", bufs=KC)
                   for _ in range(KC)]
            # normalized attention output, queries on partitions: col h*64+d
            out_qc = [outp.tile([128, C], BF16, tag="oqc", name="oqc_sb", bufs=NQ // 128)
                      for _ in range(NQ // 128)]
            # transposed back: channels on partitions
            outT = [outp.tile([128, NQ], BF16, tag="outT", name="outT_sb", bufs=CC) for _ in range(CC)]
            lgmax = lgp.tile([128, NCLS], F32, tag="lgmax")

            # ---- input DMA (ordered for the pipeline lead-in) ----
            # (a) query-half of xT + pair-0 q/k weight columns -> q + half of k
            for c in range(CC):
                sl = slice(c * 128, (c + 1) * 128)
                nc.sync.dma_start(out=xT[c][:, 0:NQ], in_=xT_d[sl, 0:NQ])
                nc.sync.dma_start(out=wqkv[c][:, 0:128], in_=wqkv_d[sl, 0:128])
                nc.sync.dma_start(out=wqkv[c][:, C:C + 128], in_=wqkv_d[sl, C:C + 128])
            nc.sync.dma_start(out=ident[:], in_=ident_d[:, :])
            # (b) key-half of xT  (completes k for pair 0)
            for c in range(CC):
                sl = slice(c * 128, (c + 1) * 128)
                nc.sync.dma_start(out=xT[c][:, NQ:N], in_=xT_d[sl, NQ:N])
            # (c) v weight columns for group 0 (pairs 0..2)
            for c in range(CC):
                sl = slice(c * 128, (c + 1) * 128)
                nc.sync.dma_start(out=wqkv[c][:, 2 * C:2 * C + VG * 128],
                                  in_=wqkv_d[sl, 2 * C:2 * C + VG * 128])
            # (d) remaining q/k weight columns + v group 1
            for c in range(CC):
                sl = slice(c * 128, (c + 1) * 128)
                nc.sync.dma_start(out=wqkv[c][:, 128:C], in_=wqkv_d[sl, 128:C])
                nc.sync.dma_start(out=wqkv[c][:, C + 128:2 * C],
                                  in_=wqkv_d[sl, C + 128:2 * C])
                nc.sync.dma_start(out=wqkv[c][:, 2 * C + VG * 128:],
                                  in_=wqkv_d[sl, 2 * C + VG * 128:])
            # (e) classifier weights
            for c in range(CC):
                sl = slice(c * 128, (c + 1) * 128)
                nc.sync.dma_start(out=wf[c][:], in_=wf_d[sl, :])

            # DoubleRow zero lanes (GPSIMD is otherwise idle here)
            for p in range(PAIRS):
                nc.gpsimd.memset(qk8[p][:, NQ:2 * NQ], 0.0)
                nc.gpsimd.memset(qk8[p][:, 2 * NQ + N:], 0.0)

            # ---- unit emitters ----
            def q_unit(p):
                ps = psp.tile([128, 1024], F32, tag="ps", name="ps")
                for c in range(CC):
                    for s0 in (0, 512):
                        nc.tensor.matmul(
                            ps[:, s0:s0 + 512],
                            lhsT=wqkv[c][:, p * 128:(p + 1) * 128],
                            rhs=xT[c][:, s0:s0 + 512],
                            start=(c == 0), stop=(c == CC - 1))
                nc.vector.tensor_copy(out=qk8[p][:, 0:NQ], in_=ps[:])

            def k_unit(p, u):
                # keys u*1024 .. (u+1)*1024
                ps = psp.tile([128, 1024], F32, tag="ps", name="ps")
                for c in range(CC):
                    for s0 in (0, 512):
                        nc.tensor.matmul(
                            ps[:, s0:s0 + 512],
                            lhsT=wqkv[c][:, C + p * 128:C + (p + 1) * 128],
                            rhs=xT[c][:, u * 1024 + s0:u * 1024 + s0 + 512],
                            start=(c == 0), stop=(c == CC - 1))
                nc.vector.tensor_copy(
                    out=qk8[p][:, 2 * NQ + u * 1024:2 * NQ + (u + 1) * 1024], in_=ps[:])

            def v_group(g, kc):
                """v65[kc] columns for pairs 3g..3g+2 (+ ones columns)."""
                w0 = 2 * C + g * VG * 2 * HD
                psb = psp.tile([128, 1024], F32, tag="ps", name="ps")
                ps = psb[:, 0:VG * 2 * HD]
                for c in range(CC):
                    nc.tensor.matmul(
                        ps[:], lhsT=xT[c][:, kc * 128:(kc + 1) * 128],
                        rhs=wqkv[c][:, w0:w0 + VG * 2 * HD],
                        start=(c == 0), stop=(c == CC - 1))
                vdst = v65[kc][:].rearrange("p (h d) -> p h d", d=HD + 1)
                hs = slice(g * VG * 2, (g + 1) * VG * 2)
                nc.vector.memset(vdst[:, hs, HD:HD + 1], 1.0)
                nc.vector.tensor_copy(
                    out=vdst[:, hs, 0:HD],
                    in_=ps[:].rearrange("p (h d) -> p h d", d=HD))

            es = {}     # (p, kc) -> (e0, e1)
            avb = {}    # p -> [3 psum bank tiles]

            def scores_step(p, kc):
                qv = qk8[p][:, 0:2 * NQ].rearrange("p (i n) -> p i n", i=2)
                kv = qk8[p][:, 2 * NQ:].rearrange("p (i n) -> p i n", i=2)
                pair_es = []
                for h in (0, 1):
                    rows = slice(h * HD, (h + 1) * HD)
                    st = psp.tile([128, 1024], F32, tag="ps", name="ps")
                    for s0 in (0, 512):
                        nc.tensor.matmul(
                            st[:, s0:s0 + 512],
                            lhsT=kv[rows, :, kc * 128:(kc + 1) * 128],
                            rhs=qv[rows, :, s0:s0 + 512],
                            start=True, stop=True, perf_mode=DR)
                    e = ep.tile([128, 1024], BF16, tag="e", name="e")
                    nc.scalar.activation(out=e[:], in_=st[:], func=EXP,
                                         scale=ESCALE)
                    pair_es.append(e)
                es[(p, kc)] = pair_es

            # av accumulator geometry: 8 slots of 130 cols over 3 banks
            # bank 0: qc 0..2, bank 1: qc 3..5, bank 2: qc 6..7
            def av_slot(qc):
                return qc // 3, (qc % 3) * 130

            def av_step(p, kc):
                if kc == 0:
                    avb[p] = [avp.tile([128, 512], F32, tag="av", name="av")
                              for _ in range(3)]
                banks = avb[p]
                pair_es = es.pop((p, kc))
                for qc in range(NQ // 128):
                    bnk, col = av_slot(qc)
                    for h in (0, 1):
                        first = kc == 0 and col == 0 and h == 0
                        last = (kc == KC - 1 and h == 1
                                and (qc % 3 == 2 or qc == NQ // 128 - 1))
                        nc.tensor.matmul(
                            banks[bnk][:, col + h * 65:col + h * 65 + 65],
                            lhsT=pair_es[h][:, qc * 128:(qc + 1) * 128],
                            rhs=v65[kc][:, (2 * p + h) * 65:(2 * p + h) * 65 + 65],
                            start=first, stop=last,
                            skip_group_check=not (first or last))

            def norm(p, qc):
                bnk, col = av_slot(qc)
                banks = avb[p]
                r = smallp.tile([128, 2], F32, tag="rcp", name="rcp")
                nc.vector.reciprocal_approx_fast(
                    out=r[:], in_=banks[bnk][:, col + 64:col + 130:65])
                for h in (0, 1):
                    nc.vector.tensor_scalar_mul(
                        out=out_qc[qc][:, (2 * p + h) * HD:(2 * p + h + 1) * HD],
                        in0=banks[bnk][:, col + h * 65:col + h * 65 + 64],
                        scalar1=r[:, h:h + 1])

            def tpose(p, qc):
                tp = tpp.tile([128, 128], BF16, tag="tp", name="tp")
                nc.tensor.transpose(
                    tp[:], in_=out_qc[qc][:, p * 128:(p + 1) * 128],
                    identity=ident[:])
                nc.vector.tensor_copy(
                    out=outT[p][:, qc * 128:(qc + 1) * 128], in_=tp[:])

            def clf(qc):
                ps = psp.tile([128, 1024], F32, tag="ps", name="ps")
                for s0 in (0, 512):
                    sw = min(512, NCLS - s0)
                    for c in range(CC):
                        nc.tensor.matmul(
                            ps[:, s0:s0 + sw],
                            lhsT=outT[c][:, qc * 128:(qc + 1) * 128],
                            rhs=wf[c][:, s0:s0 + sw],
                            start=(c == 0), stop=(c == CC - 1))
                if qc == 0:
                    nc.vector.tensor_copy(out=lgmax[:], in_=ps[:, 0:NCLS])
                else:
                    nc.vector.tensor_max(out=lgmax[:], in0=ps[:, 0:NCLS],
                                         in1=lgmax[:])

            # ---- software-pipelined emission ----
            # extras[s]: filler callables interleaved at global step s
            extras = {}

            def add_extra(s, f):
                extras.setdefault(s, []).append(f)

            # pair 0 q/k projection is the lead-in (before step 0);
            # later pairs' projections ride as fillers.
            for p in range(1, PAIRS):
                base = (p - 1) * KC
                add_extra(base + 4, lambda p=p: q_unit(p))
                add_extra(base + 8, lambda p=p: k_unit(p, 0))
                add_extra(base + 12, lambda p=p: k_unit(p, 1))
            # v group 0 rides inside pair 0 (consumed LAG steps later);
            # group 1 spreads over pairs 1-2.
            for kc in range(KC):
                add_extra(kc, lambda kc=kc: v_group(0, kc))
                add_extra(KC + 2 * kc, lambda kc=kc: v_group(1, kc))

            q_unit(0)
            k_unit(0, 0)
            k_unit(0, 1)

            total = PAIRS * KC
            for s in range(total + LAG):
                if s < total:
                    p, kc = divmod(s, KC)
                    scores_step(p, kc)
                t = s - LAG
                if t >= 0:
                    ap_, akc = divmod(t, KC)
                    av_step(ap_, akc)
                    if akc == KC - 1:
                        for qc in range(NQ // 128):
                            norm(ap_, qc)
                        if ap_ < PAIRS - 1:
                            # spread the pair's transposes over upcoming steps
                            for qc in range(NQ // 128):
                                add_extra(s + 1 + qc // 2,
                                          lambda ap_=ap_, qc=qc: tpose(ap_, qc))
                for f in extras.pop(s, ()):
                    f()

            # tail: last pair's transposes feed the classifier per qc
            for qc in range(NQ // 128):
                tpose(PAIRS - 1, qc)
                clf(qc)

            # final 128-way partition max happens on the host
            nc.sync.dma_start(out=out_d[:, :], in_=lgmax[:])

    nc.compile()
    return nc


def _prep_inputs(x, w_qkv, w_proj, b_proj, w_head, b_head):
    bf = ml_dtypes.bfloat16
    x = np.asarray(x, dtype=np.float32)
    w_qkv = np.asarray(w_qkv, np.float32).copy()
    # fold the fp8 prescale into the q/k weight columns
    w_qkv[:, 0:2 * C] *= QK_PRESCALE
    wf = (np.asarray(w_proj, np.float64) @ np.asarray(w_head, np.float64))
    wf = wf.astype(np.float32)
    b_const = (np.asarray(b_proj, np.float32) @ np.asarray(w_head, np.float32)
               + np.asarray(b_head, np.float32))

    wqkv_b = np.ascontiguousarray(w_qkv.astype(bf))
    wf_b = np.ascontiguousarray(wf.astype(bf))
    ident = np.eye(128, dtype=bf)
    in_maps = []
    for core in range(8):
        b, half = core // 2, core % 2
        xb = x[b] if half == 0 else np.concatenate(
            [x[b, NQ:], x[b, :NQ]], axis=0)   # rotate keys: own queries first
        xTb = np.ascontiguousarray(xb.T.astype(bf))                # [768, 2048]
        in_maps.append({"xT": xTb, "wqkv": wqkv_b, "wf": wf_b, "ident": ident})
    return in_maps, b_const


def kernel(x, w_qkv, w_proj, b_proj, w_head, b_head):
    if "nc" not in _CACHE:
        _CACHE["nc"] = _build()
    nc = _CACHE["nc"]

    in_maps, b_const = _prep_inputs(x, w_qkv, w_proj, b_proj, w_head, b_head)
    res = run_bass_kernel_spmd(nc, in_maps, core_ids=list(range(8)))

    out = np.empty((B, NUM_CLASSES), np.float32)
    for b in range(B):
        lo = res.results[2 * b]["out"].max(axis=0)
        hi = res.results[2 * b + 1]["out"].max(axis=0)
        out[b] = np.maximum(lo, hi)[:NUM_CLASSES] + b_const
    return out


if __name__ == "__main__":
    sys.path.insert(0, "/root/problem")
    import reference

    inputs = {k: np.asarray(v) for k, v in reference.setup_inputs().items()}
    expected = np.asarray(reference.reference(**inputs))
    actual = kernel(**inputs)
    num = np.linalg.norm(actual - expected)
    den = np.linalg.norm(expected)
    print("rel fro err:", num / den)
